# revision 16
# baseline (speedup 1.0000x reference)
"""Int8RouterLinear TRN2 kernel: out[16384, 64] = x[16384, 4096] @ (W_int8 * scale)^T.

v2 strategy (data-parallel over 8 NeuronCores, 2048 tokens each):
  - Host quantizes x per token: h-tiles k>=8 to int8 (u = rint(x/s_t),
    s_t = absmax_t/127), h-tiles k<8 to fp8-e4m3 of x/s_t. 1 byte/elem
    either way -> 8MB of x per core (vs 14.1MB for the fp16/fp8 mix).
    int8's uniform grid is ~3x more accurate than fp8 for Gaussian x.
  - On device, int8 h-tiles are cast to fp16 (exact: |u| <= 127) split
    across DVE (2x mode, ~1.92 elem/ns/partition) and ACT
    ((N+352)/1.2ns); fp8 tiles feed the PE directly (fp16 lhsT x fp8
    rhs mixed matmul, same speed).
  - PE runs col-tiled: the 2048 tokens form 2 super-chunks of 1024; a
    super-chunk's two 512-token halves run CONCURRENTLY in PE column
    groups 0-63 / 64-127 (tile_position via out base partition), so a
    k-step costs ~216ns for 1024 tokens -> ~14us PE total.
  - PSUM: one [128, 512] f32 bank per super-chunk (half-partitions =
    token halves), accumulated over the 32 h-tiles, then one ACT
    scaled-copy (2^-6, fits fp16) -> [128, 512] fp16 out, DMA'd out.
  - Host post-scales: out = psum_fp16 * 2^6 * s_t * scale_e. Weight
    ships as fp16 (int8 values exact).
  - DMA: x + w + out = 8.75MB/core over both HWDGE rings, blocks
    interleaved in program (k) order so completion tracks the
    cast/matmul consumption order.
"""
import numpy as np

import concourse.mybir as mybir
from concourse import bacc
from concourse.tile import TileContext
from concourse.bass_utils import run_bass_kernel_spmd

TOKENS = 16384
HIDDEN = 4096
EXPERTS = 64
NCORES = 8
TSHARD = TOKENS // NCORES          # 2048 tokens per core
HT = HIDDEN // 128                 # 32 h-tiles of 128
HT8 = 8                            # leading h-tiles in fp8 (no cast)
HTI = HT - HT8                     # trailing h-tiles shipped as int8
NS = 2                             # super-chunks of 1024 tokens
SU = 1024                          # tokens per super-chunk
CH = 512                           # tokens per col-group chunk

F32 = mybir.dt.float32
F16 = mybir.dt.float16
F8 = mybir.dt.float8e4
I8 = mybir.dt.int8

# DMA blocks: (name, ring, kind, u0, nu) with u0 an ABSOLUTE unit index
# into x8_d (f8: s*8+k) or xi_d (i8: s*24+(k-8)); i8 blocks may cross
# the super-chunk boundary. Constraints learned from traces:
#  - HWDGE ring depth ~4 in-flight transfers; dispatch #5+ waits a
#    completion, so blocks must be big enough to keep the wire fed.
#  - Ring1 dispatches share the ACT sequencer FIFO with ACT casts: ring1
#    gets ONLY 4 upfront transfers and nothing mid-stream.
#  - Tiny tail blocks so the last receipt gates minimal work.
# fp8 h-tile positions per super-chunk: s0 tokens quantize h-tiles 0-7
# to fp8 (PE-direct at kernel START), s1 tokens h-tiles 24-31 (PE-direct
# at the kernel TAIL -> the last-arriving blocks need no cast). Error
# budget identical: every token has exactly 8 fp8 + 24 int8 h-tiles.
F8K = [list(range(0, 8)), list(range(24, 32))]
I8K = [list(range(8, 32)), list(range(0, 24))]
# PE accumulation order per super-chunk (start on first, stop on last):
PROG = [
    F8K[0] + I8K[0],
    list(range(0, 16)) + list(range(24, 30)) + list(range(16, 24)) + [30, 31],
]
BLOCKS = [
    ("a0",  0, "i8", 0, 2),    # s0 k8-9 (DVE start)
    ("w1",  1, "w",  0, 0),
    ("b1",  1, "f8", 0, 8),    # f8 s0 (PE start)
    ("c0",  0, "i8", 2, 8),    # s0 k10-17
    ("d1",  1, "i8", 10, 8),   # s0 k18-25
    ("e0",  0, "i8", 18, 8),   # s0 k26-31 + s1 k0-1
    ("g0",  0, "i8", 26, 8),   # s1 k2-9
    ("h1",  1, "i8", 34, 6),   # s1 k10-15
    ("f2",  0, "f8", 8, 6),    # f8 s1 k24-29 (mid)
    ("i1",  1, "i8", 40, 4),   # s1 k16-19
    ("j0",  0, "i8", 44, 4),   # s1 k20-23
    ("f3",  0, "f8", 14, 2),   # f8 s1 k30-31 (last: receipt -> MM direct)
]
# int8 cast ops: (engine, s, k0, nk) in consumption order; each op reads
# within one landed block. DVE 32u@2x + w + s1 tail copy; ACT 16u@1x +
# s0 tail copy. "MID" marks where s0's MMs + ACT tail copy are emitted
# (by then s0's MMs have no pending deps -> no head-of-line stall).
CASTS = [
    ("v", 0, 8, 2),
    ("v", 0, 10, 5), ("a", 0, 15, 3),
    ("v", 0, 18, 5), ("a", 0, 23, 3),
    ("v", 0, 26, 4), ("a", 0, 30, 2),
    ("v", 1, 0, 2),
    ("v", 1, 2, 5), ("a", 1, 7, 3),
    ("MID", 0, 0, 0),
    ("v", 1, 10, 3), ("a", 1, 13, 3),
    ("v", 1, 16, 3), ("a", 1, 19, 1),
    ("v", 1, 20, 3), ("a", 1, 23, 1),
]

_cache = {}


def _build():
    if "nc" in _cache:
        return _cache["nc"]

    nc = bacc.Bacc("TRN2", target_bir_lowering=False, debug=False,
                   num_devices=NCORES)
    x8_d = nc.dram_tensor("x8", [128, NS * HT8, SU], F8, kind="ExternalInput")
    xi_d = nc.dram_tensor("xi", [128, NS * HTI, SU], I8, kind="ExternalInput")
    w_d = nc.dram_tensor("w", [128, HT * EXPERTS], I8, kind="ExternalInput")
    o_d = nc.dram_tensor("out", [128, NS * CH], F16, kind="ExternalOutput")

    with TileContext(nc) as tc:
        with tc.tile_pool(name="consts", bufs=1) as cpool, \
             tc.tile_pool(name="xp", bufs=1) as xpool, \
             tc.tile_pool(name="xf", bufs=1) as fpool, \
             tc.tile_pool(name="pso", bufs=1, space="PSUM") as ppool, \
             tc.tile_pool(name="ost", bufs=1) as opool:
            rings = [nc.sync, nc.scalar]

            src_tiles = {}      # (kind, absolute unit) -> (tile, j)
            w_i8 = cpool.tile([128, HT * EXPERTS], I8)
            w_sb = cpool.tile([128, HT * EXPERTS], F16)

            for name, ring, kind, u0, nu in BLOCKS:
                if kind == "w":
                    rings[ring].dma_start(out=w_i8, in_=w_d.ap())
                    continue
                src_d = x8_d if kind == "f8" else xi_d
                dt = F8 if kind == "f8" else I8
                t = xpool.tile([128, nu * SU], dt, name=name, tag=name)
                rings[ring].dma_start(out=t, in_=src_d.ap()[:, u0:u0 + nu, :])
                for j in range(nu):
                    src_tiles[(kind, u0 + j)] = (t, j)

            def unit(s, k):
                if k in F8K[s]:
                    return src_tiles[("f8", s * HT8 + F8K[s].index(k))]
                return src_tiles[("i8", s * HTI + I8K[s].index(k))]

            nc.vector.tensor_copy(w_sb, w_i8)   # DVE's first op, ~1.1us
            w_v = w_sb.rearrange("p (k e) -> p k e", e=EXPERTS)

            pps = [ppool.tile([128, CH], F32, name=f"pp{s}", tag=f"pp{s}")
                   for s in range(NS)]
            ots = [opool.tile([128, CH], F16, name=f"ot{s}", tag=f"ot{s}")
                   for s in range(NS)]

            def mm_superchunk(s):
                pp = pps[s]
                prog = PROG[s]
                for i, k in enumerate(prog):
                    t, j = unit(s, k) if k in F8K[s] else f16_units[(s, k)]
                    tv = t.rearrange("p (u t) -> p u t", t=SU)
                    wt = w_v[:, k, :]
                    nc.tensor.matmul(pp[0:64, :], wt, tv[:, j, 0:CH],
                                     start=(i == 0), stop=(i == HT - 1))
                    nc.tensor.matmul(pp[64:128, :], wt, tv[:, j, CH:SU],
                                     start=(i == 0), stop=(i == HT - 1))
                # tail: one scaled fp32->fp16 copy (2^-6 keeps |v| < 2^16/6.4)
                # s0 on ACT (inserted mid-FIFO), s1 on DVE (idle at the end).
                if s == 0:
                    nc.scalar.mul(ots[s], pp, 0.015625)
                else:
                    nc.vector.tensor_scalar_mul(ots[s], pp, 0.015625)
                rings[0].dma_start(out=o_d.ap()[:, s * CH:(s + 1) * CH],
                                   in_=ots[s])

            # cast int8 units -> fp16 tiles; at MID, emit all of s0's MMs
            # + its ACT tail copy so the copy sits early in the ACT FIFO.
            f16_units = {}
            for eng, s, k0, nk in CASTS:
                if eng == "MID":
                    mm_superchunk(0)
                    continue
                it, j0 = unit(s, k0)
                iv = it.rearrange("p (u t) -> p u t", t=SU)
                ft = fpool.tile([128, nk * SU], F16, name=f"c{eng}{s}k{k0}",
                                tag=f"c{eng}{s}k{k0}")
                if eng == "v":
                    nc.vector.tensor_copy(ft, iv[:, j0:j0 + nk, :])
                else:
                    nc.scalar.copy(ft, iv[:, j0:j0 + nk, :])
                for j in range(nk):
                    f16_units[(s, k0 + j)] = (ft, j)
            mm_superchunk(1)

    nc.compile()
    _cache["nc"] = nc
    return nc


def _prep_w(weights_int8, scales):
    """[64, 4096] int8-valued weights -> [128, HT*EXPERTS] int8 with
    w_arr[p, k*64 + e] = W[e, 128k + p] (cast to fp16 on device)."""
    wt = weights_int8.astype(np.int8).T                        # [H, E]
    arr = wt.reshape(HT, 128, EXPERTS).transpose(1, 0, 2)
    return np.ascontiguousarray(arr).reshape(128, HT * EXPERTS)


def _prep_x(x):
    """Quantize + transpose x into per-core (x8, xi) plus token scales.
    Per super-chunk s, h-tiles F8K[s] ship as fp8(x/s_tok) and I8K[s] as
    rint(x/s_tok) int8:
      x8[p, s*8+i,  t] = fp8(xs[T0 + s*1024 + t, 128*F8K[s][i] + p])
      xi[p, s*24+i, t] = rint(xs[T0 + s*1024 + t, 128*I8K[s][i] + p])
    """
    f8np = mybir.dt.np(F8)
    s_tok = np.abs(x).max(axis=1) / 127.0            # [TOKENS]
    s_tok = np.maximum(s_tok, 1e-12).astype(np.float32)
    xs = x / s_tok[:, None]                          # |xs| <= 127
    x8 = xs.astype(f8np)
    xi = np.clip(np.rint(xs), -127, 127).astype(np.int8)
    xt8 = np.empty((HIDDEN, TOKENS), dtype=f8np)
    xti = np.empty((HIDDEN, TOKENS), dtype=np.int8)
    blk = 512
    for i in range(0, TOKENS, blk):
        xt8[:, i:i + blk] = x8[i:i + blk].T
        xti[:, i:i + blk] = xi[i:i + blk].T
    f8rows = [(F8K[s][0] * 128, (F8K[s][-1] + 1) * 128) for s in range(NS)]
    i8rows = [(I8K[s][0] * 128, (I8K[s][-1] + 1) * 128) for s in range(NS)]
    shards = []
    for c in range(NCORES):
        a8 = np.empty((128, NS * HT8, SU), dtype=f8np)
        ai = np.empty((128, NS * HTI, SU), dtype=np.int8)
        for s in range(NS):
            sl = slice(c * TSHARD + s * SU, c * TSHARD + (s + 1) * SU)
            r0, r1 = f8rows[s]
            a8[:, s * HT8:(s + 1) * HT8, :] = (
                xt8[r0:r1, sl].reshape(HT8, 128, SU).transpose(1, 0, 2))
            r0, r1 = i8rows[s]
            ai[:, s * HTI:(s + 1) * HTI, :] = (
                xti[r0:r1, sl].reshape(HTI, 128, SU).transpose(1, 0, 2))
        shards.append((np.ascontiguousarray(a8), np.ascontiguousarray(ai)))
    return shards, s_tok


def kernel(x, weights_int8, scales):
    nc = _build()
    x = np.ascontiguousarray(np.asarray(x), dtype=np.float32)
    warr = _prep_w(np.asarray(weights_int8), np.asarray(scales))
    shards, s_tok = _prep_x(x)
    in_maps = [{"x8": shards[c][0], "xi": shards[c][1], "w": warr}
               for c in range(NCORES)]
    res = run_bass_kernel_spmd(nc, in_maps, core_ids=list(range(NCORES)))
    scales_f = np.asarray(scales, dtype=np.float64)
    out = np.empty((TOKENS, EXPERTS), dtype=np.float64)
    for c in range(NCORES):
        o = res.results[c]["out"].astype(np.float64)      # [128, NS*CH]
        o = o.reshape(2, 64, NS, CH)                      # [chunk, e, s, t]
        for s in range(NS):
            for ch in range(2):
                t0 = c * TSHARD + s * SU + ch * CH
                out[t0:t0 + CH] = o[ch, :, s, :].T
    out *= 64.0 * s_tok[:, None].astype(np.float64)
    out *= scales_f[None, :]
    return np.ascontiguousarray(out, dtype=np.float32)


# revision 18
# speedup vs baseline: 1.0002x; 1.0002x over previous
"""Int8RouterLinear TRN2 kernel: out[16384, 64] = x[16384, 4096] @ (W_int8 * scale)^T.

v2 strategy (data-parallel over 8 NeuronCores, 2048 tokens each):
  - Host quantizes x per token: h-tiles k>=8 to int8 (u = rint(x/s_t),
    s_t = absmax_t/127), h-tiles k<8 to fp8-e4m3 of x/s_t. 1 byte/elem
    either way -> 8MB of x per core (vs 14.1MB for the fp16/fp8 mix).
    int8's uniform grid is ~3x more accurate than fp8 for Gaussian x.
  - On device, int8 h-tiles are cast to fp16 (exact: |u| <= 127) split
    across DVE (2x mode, ~1.92 elem/ns/partition) and ACT
    ((N+352)/1.2ns); fp8 tiles feed the PE directly (fp16 lhsT x fp8
    rhs mixed matmul, same speed).
  - PE runs col-tiled: the 2048 tokens form 2 super-chunks of 1024; a
    super-chunk's two 512-token halves run CONCURRENTLY in PE column
    groups 0-63 / 64-127 (tile_position via out base partition), so a
    k-step costs ~216ns for 1024 tokens -> ~14us PE total.
  - PSUM: one [128, 512] f32 bank per super-chunk (half-partitions =
    token halves), accumulated over the 32 h-tiles, then one ACT
    scaled-copy (2^-6, fits fp16) -> [128, 512] fp16 out, DMA'd out.
  - Host post-scales: out = psum_fp16 * 2^6 * s_t * scale_e. Weight
    ships as fp16 (int8 values exact).
  - DMA: x + w + out = 8.75MB/core over both HWDGE rings, blocks
    interleaved in program (k) order so completion tracks the
    cast/matmul consumption order.
"""
import numpy as np

import concourse.mybir as mybir
from concourse import bacc
from concourse.tile import TileContext
from concourse.bass_utils import run_bass_kernel_spmd

TOKENS = 16384
HIDDEN = 4096
EXPERTS = 64
NCORES = 8
TSHARD = TOKENS // NCORES          # 2048 tokens per core
HT = HIDDEN // 128                 # 32 h-tiles of 128
HT8 = 8                            # leading h-tiles in fp8 (no cast)
HTI = HT - HT8                     # trailing h-tiles shipped as int8
NS = 2                             # super-chunks of 1024 tokens
SU = 1024                          # tokens per super-chunk
CH = 512                           # tokens per col-group chunk

F32 = mybir.dt.float32
F16 = mybir.dt.float16
F8 = mybir.dt.float8e4
I8 = mybir.dt.int8

# DMA blocks: (name, ring, kind, u0, nu) with u0 an ABSOLUTE unit index
# into x8_d (f8: s*8+k) or xi_d (i8: s*24+(k-8)); i8 blocks may cross
# the super-chunk boundary. Constraints learned from traces:
#  - HWDGE ring depth ~4 in-flight transfers; dispatch #5+ waits a
#    completion, so blocks must be big enough to keep the wire fed.
#  - Ring1 dispatches share the ACT sequencer FIFO with ACT casts: ring1
#    gets ONLY 4 upfront transfers and nothing mid-stream.
#  - Tiny tail blocks so the last receipt gates minimal work.
# fp8 h-tile positions per super-chunk: s0 tokens quantize h-tiles 0-7
# to fp8 (PE-direct at kernel START), s1 tokens h-tiles 24-31 (PE-direct
# at the kernel TAIL -> the last-arriving blocks need no cast). Error
# budget identical: every token has exactly 8 fp8 + 24 int8 h-tiles.
F8K = [list(range(0, 8)), list(range(24, 32))]
I8K = [list(range(8, 32)), list(range(0, 24))]
# PE accumulation order per super-chunk (start on first, stop on last):
PROG = [
    F8K[0] + I8K[0],
    list(range(0, 16)) + list(range(24, 30)) + list(range(16, 24)) + [30, 31],
]
BLOCKS = [
    ("a0",  0, "i8", 0, 2),    # s0 k8-9 (DVE start)
    ("w1",  1, "w",  0, 0),
    ("b1",  1, "f8", 0, 8),    # f8 s0 (PE start)
    ("c0",  0, "i8", 2, 8),    # s0 k10-17
    ("d1",  1, "i8", 10, 8),   # s0 k18-25
    ("e0",  0, "i8", 18, 8),   # s0 k26-31 + s1 k0-1
    ("g0",  0, "i8", 26, 8),   # s1 k2-9
    ("h1",  1, "i8", 34, 6),   # s1 k10-15
    ("f2",  0, "f8", 8, 6),    # f8 s1 k24-29 (mid)
    ("i1",  1, "i8", 40, 4),   # s1 k16-19
    ("j0",  0, "i8", 44, 4),   # s1 k20-23
    ("f3",  0, "f8", 14, 2),   # f8 s1 k30-31 (last: receipt -> MM direct)
]
# Emission script. Interleaving MM emission with cast emission matters:
# the Tile scheduler coalesces an engine's semaphore waits based on its
# (DMA-latency-blind) sim, so MMs emitted after unrelated later casts
# pick up false dependencies on them and stall the PE for real. Ops:
#   ("cast", eng, s, k0, nk)  int8 h-tiles k0..k0+nk-1 of super-chunk s
#   ("mm", s, i0, ni)         PROG[s][i0..i0+ni-1] matmul pairs
#   ("tail", s)               scaled PSUM->fp16 copy + out store
# DVE 32 cast units @2x + w + s1 tail; ACT 16 units @1x + s0 tail.
EMIT = [
    ("cast", "v", 0, 8, 2), ("mm", 0, 0, 10),
    ("cast", "v", 0, 10, 5), ("cast", "a", 0, 15, 3), ("mm", 0, 10, 8),
    ("cast", "v", 0, 18, 5), ("cast", "a", 0, 23, 3), ("mm", 0, 18, 8),
    ("cast", "v", 0, 26, 4), ("cast", "a", 0, 30, 2), ("mm", 0, 26, 6),
    ("cast", "v", 1, 0, 2), ("cast", "v", 1, 2, 5), ("cast", "a", 1, 7, 3),
    ("mm", 1, 0, 10),
    ("tail", 0),
    ("cast", "v", 1, 10, 3), ("cast", "a", 1, 13, 3), ("mm", 1, 10, 12),
    ("cast", "v", 1, 16, 3), ("cast", "a", 1, 19, 1), ("mm", 1, 22, 4),
    ("cast", "v", 1, 20, 3), ("cast", "a", 1, 23, 1), ("mm", 1, 26, 6),
    ("tail", 1),
]

_cache = {}


def _build():
    if "nc" in _cache:
        return _cache["nc"]

    nc = bacc.Bacc("TRN2", target_bir_lowering=False, debug=False,
                   num_devices=NCORES)
    x8_d = nc.dram_tensor("x8", [128, NS * HT8, SU], F8, kind="ExternalInput")
    xi_d = nc.dram_tensor("xi", [128, NS * HTI, SU], I8, kind="ExternalInput")
    w_d = nc.dram_tensor("w", [128, HT * EXPERTS], I8, kind="ExternalInput")
    o_d = nc.dram_tensor("out", [128, NS * CH], F16, kind="ExternalOutput")

    with TileContext(nc) as tc:
        with tc.tile_pool(name="consts", bufs=1) as cpool, \
             tc.tile_pool(name="xp", bufs=1) as xpool, \
             tc.tile_pool(name="xf", bufs=1) as fpool, \
             tc.tile_pool(name="pso", bufs=1, space="PSUM") as ppool, \
             tc.tile_pool(name="ost", bufs=1) as opool:
            rings = [nc.sync, nc.scalar]

            src_tiles = {}      # (kind, absolute unit) -> (tile, j)
            w_i8 = cpool.tile([128, HT * EXPERTS], I8)
            w_sb = cpool.tile([128, HT * EXPERTS], F16)

            for name, ring, kind, u0, nu in BLOCKS:
                if kind == "w":
                    rings[ring].dma_start(out=w_i8, in_=w_d.ap())
                    continue
                src_d = x8_d if kind == "f8" else xi_d
                dt = F8 if kind == "f8" else I8
                t = xpool.tile([128, nu * SU], dt, name=name, tag=name)
                rings[ring].dma_start(out=t, in_=src_d.ap()[:, u0:u0 + nu, :])
                for j in range(nu):
                    src_tiles[(kind, u0 + j)] = (t, j)

            def unit(s, k):
                if k in F8K[s]:
                    return src_tiles[("f8", s * HT8 + F8K[s].index(k))]
                return src_tiles[("i8", s * HTI + I8K[s].index(k))]

            nc.vector.tensor_copy(w_sb, w_i8)   # DVE's first op, ~1.1us
            w_v = w_sb.rearrange("p (k e) -> p k e", e=EXPERTS)

            pps = [ppool.tile([128, CH], F32, name=f"pp{s}", tag=f"pp{s}")
                   for s in range(NS)]
            ots = [opool.tile([128, CH], F16, name=f"ot{s}", tag=f"ot{s}")
                   for s in range(NS)]

            f16_units = {}
            for op in EMIT:
                if op[0] == "cast":
                    _, eng, s, k0, nk = op
                    it, j0 = unit(s, k0)
                    iv = it.rearrange("p (u t) -> p u t", t=SU)
                    ft = fpool.tile([128, nk * SU], F16,
                                    name=f"c{eng}{s}k{k0}",
                                    tag=f"c{eng}{s}k{k0}")
                    if eng == "v":
                        nc.vector.tensor_copy(ft, iv[:, j0:j0 + nk, :])
                    else:
                        nc.scalar.copy(ft, iv[:, j0:j0 + nk, :])
                    for j in range(nk):
                        f16_units[(s, k0 + j)] = (ft, j)
                elif op[0] == "mm":
                    _, s, i0, ni = op
                    pp = pps[s]
                    for i in range(i0, i0 + ni):
                        k = PROG[s][i]
                        t, j = unit(s, k) if k in F8K[s] else f16_units[(s, k)]
                        tv = t.rearrange("p (u t) -> p u t", t=SU)
                        wt = w_v[:, k, :]
                        nc.tensor.matmul(pp[0:64, :], wt, tv[:, j, 0:CH],
                                         start=(i == 0), stop=(i == HT - 1))
                        nc.tensor.matmul(pp[64:128, :], wt, tv[:, j, CH:SU],
                                         start=(i == 0), stop=(i == HT - 1))
                else:
                    # tail: scaled fp32->fp16 copy (2^-6 keeps |v| < 2^16/6.4)
                    # s0 on ACT (mid-FIFO), s1 on DVE (idle at the end).
                    _, s = op
                    if s == 0:
                        nc.scalar.mul(ots[s], pps[s], 0.015625)
                    else:
                        nc.vector.tensor_scalar_mul(ots[s], pps[s], 0.015625)
                    rings[0].dma_start(out=o_d.ap()[:, s * CH:(s + 1) * CH],
                                       in_=ots[s])

    nc.compile()
    _cache["nc"] = nc
    return nc


def _prep_w(weights_int8, scales):
    """[64, 4096] int8-valued weights -> [128, HT*EXPERTS] int8 with
    w_arr[p, k*64 + e] = W[e, 128k + p] (cast to fp16 on device)."""
    wt = weights_int8.astype(np.int8).T                        # [H, E]
    arr = wt.reshape(HT, 128, EXPERTS).transpose(1, 0, 2)
    return np.ascontiguousarray(arr).reshape(128, HT * EXPERTS)


def _prep_x(x):
    """Quantize + transpose x into per-core (x8, xi) plus token scales.
    Per super-chunk s, h-tiles F8K[s] ship as fp8(x/s_tok) and I8K[s] as
    rint(x/s_tok) int8:
      x8[p, s*8+i,  t] = fp8(xs[T0 + s*1024 + t, 128*F8K[s][i] + p])
      xi[p, s*24+i, t] = rint(xs[T0 + s*1024 + t, 128*I8K[s][i] + p])
    """
    f8np = mybir.dt.np(F8)
    s_tok = np.abs(x).max(axis=1) / 127.0            # [TOKENS]
    s_tok = np.maximum(s_tok, 1e-12).astype(np.float32)
    xs = x / s_tok[:, None]                          # |xs| <= 127
    x8 = xs.astype(f8np)
    xi = np.clip(np.rint(xs), -127, 127).astype(np.int8)
    xt8 = np.empty((HIDDEN, TOKENS), dtype=f8np)
    xti = np.empty((HIDDEN, TOKENS), dtype=np.int8)
    blk = 512
    for i in range(0, TOKENS, blk):
        xt8[:, i:i + blk] = x8[i:i + blk].T
        xti[:, i:i + blk] = xi[i:i + blk].T
    f8rows = [(F8K[s][0] * 128, (F8K[s][-1] + 1) * 128) for s in range(NS)]
    i8rows = [(I8K[s][0] * 128, (I8K[s][-1] + 1) * 128) for s in range(NS)]
    shards = []
    for c in range(NCORES):
        a8 = np.empty((128, NS * HT8, SU), dtype=f8np)
        ai = np.empty((128, NS * HTI, SU), dtype=np.int8)
        for s in range(NS):
            sl = slice(c * TSHARD + s * SU, c * TSHARD + (s + 1) * SU)
            r0, r1 = f8rows[s]
            a8[:, s * HT8:(s + 1) * HT8, :] = (
                xt8[r0:r1, sl].reshape(HT8, 128, SU).transpose(1, 0, 2))
            r0, r1 = i8rows[s]
            ai[:, s * HTI:(s + 1) * HTI, :] = (
                xti[r0:r1, sl].reshape(HTI, 128, SU).transpose(1, 0, 2))
        shards.append((np.ascontiguousarray(a8), np.ascontiguousarray(ai)))
    return shards, s_tok


def kernel(x, weights_int8, scales):
    nc = _build()
    x = np.ascontiguousarray(np.asarray(x), dtype=np.float32)
    warr = _prep_w(np.asarray(weights_int8), np.asarray(scales))
    shards, s_tok = _prep_x(x)
    in_maps = [{"x8": shards[c][0], "xi": shards[c][1], "w": warr}
               for c in range(NCORES)]
    res = run_bass_kernel_spmd(nc, in_maps, core_ids=list(range(NCORES)))
    scales_f = np.asarray(scales, dtype=np.float64)
    out = np.empty((TOKENS, EXPERTS), dtype=np.float64)
    for c in range(NCORES):
        o = res.results[c]["out"].astype(np.float64)      # [128, NS*CH]
        o = o.reshape(2, 64, NS, CH)                      # [chunk, e, s, t]
        for s in range(NS):
            for ch in range(2):
                t0 = c * TSHARD + s * SU + ch * CH
                out[t0:t0 + CH] = o[ch, :, s, :].T
    out *= 64.0 * s_tok[:, None].astype(np.float64)
    out *= scales_f[None, :]
    return np.ascontiguousarray(out, dtype=np.float32)


# revision 22
# speedup vs baseline: 1.0718x; 1.0716x over previous
"""Int8RouterLinear TRN2 kernel: out[16384, 64] = x[16384, 4096] @ (W_int8 * scale)^T.

v2 strategy (data-parallel over 8 NeuronCores, 2048 tokens each):
  - Host quantizes x per token: h-tiles k>=8 to int8 (u = rint(x/s_t),
    s_t = absmax_t/127), h-tiles k<8 to fp8-e4m3 of x/s_t. 1 byte/elem
    either way -> 8MB of x per core (vs 14.1MB for the fp16/fp8 mix).
    int8's uniform grid is ~3x more accurate than fp8 for Gaussian x.
  - On device, int8 h-tiles are cast to fp16 (exact: |u| <= 127) split
    across DVE (2x mode, ~1.92 elem/ns/partition) and ACT
    ((N+352)/1.2ns); fp8 tiles feed the PE directly (fp16 lhsT x fp8
    rhs mixed matmul, same speed).
  - PE runs col-tiled: the 2048 tokens form 2 super-chunks of 1024; a
    super-chunk's two 512-token halves run CONCURRENTLY in PE column
    groups 0-63 / 64-127 (tile_position via out base partition), so a
    k-step costs ~216ns for 1024 tokens -> ~14us PE total.
  - PSUM: one [128, 512] f32 bank per super-chunk (half-partitions =
    token halves), accumulated over the 32 h-tiles, then one ACT
    scaled-copy (2^-6, fits fp16) -> [128, 512] fp16 out, DMA'd out.
  - Host post-scales: out = psum_fp16 * 2^6 * s_t * scale_e. Weight
    ships as fp16 (int8 values exact).
  - DMA: x + w + out = 8.75MB/core over both HWDGE rings, blocks
    interleaved in program (k) order so completion tracks the
    cast/matmul consumption order.
"""
import numpy as np

import concourse.mybir as mybir
from concourse import bacc
from concourse.tile import TileContext
from concourse.tile_rust import add_dep_helper
from concourse.bass_utils import run_bass_kernel_spmd

TOKENS = 16384
HIDDEN = 4096
EXPERTS = 64
NCORES = 8
TSHARD = TOKENS // NCORES          # 2048 tokens per core
HT = HIDDEN // 128                 # 32 h-tiles of 128
HT8 = 8                            # leading h-tiles in fp8 (no cast)
HTI = HT - HT8                     # trailing h-tiles shipped as int8
NS = 2                             # super-chunks of 1024 tokens
SU = 1024                          # tokens per super-chunk
CH = 512                           # tokens per col-group chunk

F32 = mybir.dt.float32
F16 = mybir.dt.float16
F8 = mybir.dt.float8e4
I8 = mybir.dt.int8

# DMA blocks: (name, ring, kind, u0, nu) with u0 an ABSOLUTE unit index
# into x8_d (f8: s*8+k) or xi_d (i8: s*24+(k-8)); i8 blocks may cross
# the super-chunk boundary. Constraints learned from traces:
#  - HWDGE ring depth ~4 in-flight transfers; dispatch #5+ waits a
#    completion, so blocks must be big enough to keep the wire fed.
#  - Ring1 dispatches share the ACT sequencer FIFO with ACT casts: ring1
#    gets ONLY 4 upfront transfers and nothing mid-stream.
#  - Tiny tail blocks so the last receipt gates minimal work.
# fp8 h-tile positions per super-chunk: s0 tokens quantize h-tiles 0-7
# to fp8 (PE-direct at kernel START), s1 tokens h-tiles 24-31 (PE-direct
# at the kernel TAIL -> the last-arriving blocks need no cast). Error
# budget identical: every token has exactly 8 fp8 + 24 int8 h-tiles.
F8K = [list(range(0, 8)), list(range(24, 32))]
I8K = [list(range(8, 32)), list(range(0, 24))]
# PE accumulation order per super-chunk (start on first, stop on last):
PROG = [
    F8K[0] + I8K[0],
    list(range(0, 16)) + list(range(24, 30)) + list(range(16, 24)) + [30, 31],
]
BLOCKS = [
    ("a0",  0, "i8", 0, 2),    # s0 k8-9 (DVE start)
    ("w1",  1, "w",  0, 0),
    ("b1",  1, "f8", 0, 8),    # f8 s0 (PE start)
    ("c0",  0, "i8", 2, 8),    # s0 k10-17
    ("d1",  1, "i8", 10, 8),   # s0 k18-25
    ("e0",  0, "i8", 18, 8),   # s0 k26-31 + s1 k0-1
    ("g0",  0, "i8", 26, 8),   # s1 k2-9
    ("h1",  1, "i8", 34, 6),   # s1 k10-15
    ("f2",  0, "f8", 8, 6),    # f8 s1 k24-29 (mid)
    ("i1",  1, "i8", 40, 4),   # s1 k16-19
    ("j0",  0, "i8", 44, 4),   # s1 k20-23
    ("f3",  0, "f8", 14, 2),   # f8 s1 k30-31 (last: receipt -> MM direct)
]
# Emission script. Interleaving MM emission with cast emission matters:
# the Tile scheduler coalesces an engine's semaphore waits based on its
# (DMA-latency-blind) sim, so MMs emitted after unrelated later casts
# pick up false dependencies on them and stall the PE for real. Ops:
#   ("cast", eng, s, k0, nk)  int8 h-tiles k0..k0+nk-1 of super-chunk s
#   ("mm", s, i0, ni)         PROG[s][i0..i0+ni-1] matmul pairs
#   ("tail", s)               scaled PSUM->fp16 copy + out store
# DVE 32 cast units @2x + w + s1 tail; ACT 16 units @1x + s0 tail.
EMIT = [
    ("cast", "v", 0, 8, 2), ("mm", 0, 0, 10),
    ("cast", "v", 0, 10, 5), ("cast", "a", 0, 15, 3), ("mm", 0, 10, 8),
    ("cast", "v", 0, 18, 5), ("cast", "a", 0, 23, 3), ("mm", 0, 18, 8),
    ("cast", "v", 0, 26, 4), ("cast", "a", 0, 30, 2), ("mm", 0, 26, 6),
    ("cast", "v", 1, 0, 2), ("cast", "v", 1, 2, 5), ("cast", "a", 1, 7, 3),
    ("mm", 1, 0, 10),
    ("tail", 0),
    ("cast", "v", 1, 10, 3), ("cast", "a", 1, 13, 3), ("mm", 1, 10, 12),
    ("cast", "v", 1, 16, 3), ("cast", "a", 1, 19, 1), ("mm", 1, 22, 4),
    ("cast", "v", 1, 20, 3), ("cast", "a", 1, 23, 1), ("mm", 1, 26, 6),
    ("tail", 1),
]

_cache = {}


def _build():
    if "nc" in _cache:
        return _cache["nc"]

    nc = bacc.Bacc("TRN2", target_bir_lowering=False, debug=False,
                   num_devices=NCORES)
    x8_d = nc.dram_tensor("x8", [128, NS * HT8, SU], F8, kind="ExternalInput")
    xi_d = nc.dram_tensor("xi", [128, NS * HTI, SU], I8, kind="ExternalInput")
    w_d = nc.dram_tensor("w", [128, HT * EXPERTS], I8, kind="ExternalInput")
    o_d = nc.dram_tensor("out", [128, NS * CH], F16, kind="ExternalOutput")

    with TileContext(nc) as tc:
        with tc.tile_pool(name="consts", bufs=1) as cpool, \
             tc.tile_pool(name="xp", bufs=1) as xpool, \
             tc.tile_pool(name="xf", bufs=1) as fpool, \
             tc.tile_pool(name="pso", bufs=1, space="PSUM") as ppool, \
             tc.tile_pool(name="ost", bufs=1) as opool:
            rings = [nc.sync, nc.scalar]

            src_tiles = {}      # (kind, absolute unit) -> (tile, j)
            w_i8 = cpool.tile([128, HT * EXPERTS], I8)
            w_sb = cpool.tile([128, HT * EXPERTS], F16)

            for name, ring, kind, u0, nu in BLOCKS:
                if kind == "w":
                    rings[ring].dma_start(out=w_i8, in_=w_d.ap())
                    continue
                src_d = x8_d if kind == "f8" else xi_d
                dt = F8 if kind == "f8" else I8
                t = xpool.tile([128, nu * SU], dt, name=name, tag=name)
                rings[ring].dma_start(out=t, in_=src_d.ap()[:, u0:u0 + nu, :])
                for j in range(nu):
                    src_tiles[(kind, u0 + j)] = (t, j)

            def unit(s, k):
                if k in F8K[s]:
                    return src_tiles[("f8", s * HT8 + F8K[s].index(k))]
                return src_tiles[("i8", s * HTI + I8K[s].index(k))]

            nc.vector.tensor_copy(w_sb, w_i8)   # DVE's first op, ~1.1us
            w_v = w_sb.rearrange("p (k e) -> p k e", e=EXPERTS)

            pps = [ppool.tile([128, CH], F32, name=f"pp{s}", tag=f"pp{s}")
                   for s in range(NS)]
            ots = [opool.tile([128, CH], F16, name=f"ot{s}", tag=f"ot{s}")
                   for s in range(NS)]

            f16_units = {}
            last_mm = None
            for op in EMIT:
                if op[0] == "cast":
                    _, eng, s, k0, nk = op
                    it, j0 = unit(s, k0)
                    iv = it.rearrange("p (u t) -> p u t", t=SU)
                    ft = fpool.tile([128, nk * SU], F16,
                                    name=f"c{eng}{s}k{k0}",
                                    tag=f"c{eng}{s}k{k0}")
                    if eng == "v":
                        nc.vector.tensor_copy(ft, iv[:, j0:j0 + nk, :])
                    else:
                        nc.scalar.copy(ft, iv[:, j0:j0 + nk, :])
                    for j in range(nk):
                        f16_units[(s, k0 + j)] = (ft, j)
                elif op[0] == "mm":
                    _, s, i0, ni = op
                    pp = pps[s]
                    first_of_chunk = True
                    for i in range(i0, i0 + ni):
                        k = PROG[s][i]
                        t, j = unit(s, k) if k in F8K[s] else f16_units[(s, k)]
                        tv = t.rearrange("p (u t) -> p u t", t=SU)
                        wt = w_v[:, k, :]
                        ma = nc.tensor.matmul(pp[0:64, :], wt, tv[:, j, 0:CH],
                                              start=(i == 0),
                                              stop=(i == HT - 1))
                        mb = nc.tensor.matmul(pp[64:128, :], wt,
                                              tv[:, j, CH:SU],
                                              start=(i == 0),
                                              stop=(i == HT - 1))
                        if first_of_chunk and last_mm is not None:
                            # same-engine ordering edges: forbid the
                            # scheduler from hoisting this chunk's MMs
                            # ahead of earlier chunks in the PE queue
                            # (its DMA-blind sim otherwise head-of-line
                            # blocks the PE on late data for ~5-8us).
                            add_dep_helper(ma.ins, last_mm, reason="pe order")
                            add_dep_helper(mb.ins, last_mm, reason="pe order")
                        first_of_chunk = False
                        last_mm = mb.ins
                else:
                    # tail: scaled fp32->fp16 copy (2^-6 keeps |v| < 2^16/6.4)
                    # s0 on ACT (mid-FIFO), s1 on DVE (idle at the end).
                    _, s = op
                    if s == 0:
                        nc.scalar.mul(ots[s], pps[s], 0.015625)
                    else:
                        nc.vector.tensor_scalar_mul(ots[s], pps[s], 0.015625)
                    rings[0].dma_start(out=o_d.ap()[:, s * CH:(s + 1) * CH],
                                       in_=ots[s])

    nc.compile()
    _cache["nc"] = nc
    return nc


def _prep_w(weights_int8, scales):
    """[64, 4096] int8-valued weights -> [128, HT*EXPERTS] int8 with
    w_arr[p, k*64 + e] = W[e, 128k + p] (cast to fp16 on device)."""
    wt = weights_int8.astype(np.int8).T                        # [H, E]
    arr = wt.reshape(HT, 128, EXPERTS).transpose(1, 0, 2)
    return np.ascontiguousarray(arr).reshape(128, HT * EXPERTS)


def _prep_x(x):
    """Quantize + transpose x into per-core (x8, xi) plus token scales.
    Per super-chunk s, h-tiles F8K[s] ship as fp8(x/s_tok) and I8K[s] as
    rint(x/s_tok) int8:
      x8[p, s*8+i,  t] = fp8(xs[T0 + s*1024 + t, 128*F8K[s][i] + p])
      xi[p, s*24+i, t] = rint(xs[T0 + s*1024 + t, 128*I8K[s][i] + p])
    """
    f8np = mybir.dt.np(F8)
    s_tok = np.abs(x).max(axis=1) / 127.0            # [TOKENS]
    s_tok = np.maximum(s_tok, 1e-12).astype(np.float32)
    xs = x / s_tok[:, None]                          # |xs| <= 127
    x8 = xs.astype(f8np)
    xi = np.clip(np.rint(xs), -127, 127).astype(np.int8)
    xt8 = np.empty((HIDDEN, TOKENS), dtype=f8np)
    xti = np.empty((HIDDEN, TOKENS), dtype=np.int8)
    blk = 512
    for i in range(0, TOKENS, blk):
        xt8[:, i:i + blk] = x8[i:i + blk].T
        xti[:, i:i + blk] = xi[i:i + blk].T
    f8rows = [(F8K[s][0] * 128, (F8K[s][-1] + 1) * 128) for s in range(NS)]
    i8rows = [(I8K[s][0] * 128, (I8K[s][-1] + 1) * 128) for s in range(NS)]
    shards = []
    for c in range(NCORES):
        a8 = np.empty((128, NS * HT8, SU), dtype=f8np)
        ai = np.empty((128, NS * HTI, SU), dtype=np.int8)
        for s in range(NS):
            sl = slice(c * TSHARD + s * SU, c * TSHARD + (s + 1) * SU)
            r0, r1 = f8rows[s]
            a8[:, s * HT8:(s + 1) * HT8, :] = (
                xt8[r0:r1, sl].reshape(HT8, 128, SU).transpose(1, 0, 2))
            r0, r1 = i8rows[s]
            ai[:, s * HTI:(s + 1) * HTI, :] = (
                xti[r0:r1, sl].reshape(HTI, 128, SU).transpose(1, 0, 2))
        shards.append((np.ascontiguousarray(a8), np.ascontiguousarray(ai)))
    return shards, s_tok


def kernel(x, weights_int8, scales):
    nc = _build()
    x = np.ascontiguousarray(np.asarray(x), dtype=np.float32)
    warr = _prep_w(np.asarray(weights_int8), np.asarray(scales))
    shards, s_tok = _prep_x(x)
    in_maps = [{"x8": shards[c][0], "xi": shards[c][1], "w": warr}
               for c in range(NCORES)]
    res = run_bass_kernel_spmd(nc, in_maps, core_ids=list(range(NCORES)))
    scales_f = np.asarray(scales, dtype=np.float64)
    out = np.empty((TOKENS, EXPERTS), dtype=np.float64)
    for c in range(NCORES):
        o = res.results[c]["out"].astype(np.float64)      # [128, NS*CH]
        o = o.reshape(2, 64, NS, CH)                      # [chunk, e, s, t]
        for s in range(NS):
            for ch in range(2):
                t0 = c * TSHARD + s * SU + ch * CH
                out[t0:t0 + CH] = o[ch, :, s, :].T
    out *= 64.0 * s_tok[:, None].astype(np.float64)
    out *= scales_f[None, :]
    return np.ascontiguousarray(out, dtype=np.float32)
